# revision 8
# baseline (speedup 1.0000x reference)
"""Trainium2 Bass kernel for nn_DecLayerJ (gnn message passing decoder layer).

Strategy (per NeuronCore, 8-way data parallel over B*N nodes):
  - Edge phase: stream h_E in 768-token chunks (16 nodes x K=48).
    SWDGE cast-load f32->bf16, xbar-DMA transpose to feature-major
    [C, tokens], W1/W2 matmuls in bf16 (f32 PSUM accum), tanh-gelu on ACT,
    mask broadcast via rank-1 PE matmul, masked K-sum via DVE reduce.
  - Node phase: S @ (W3/30) + (sum_k mask)*b3/30, residual, FFN with exact
    gelu, mask_V, transpose back to token-major, store.
h_V residual path stays fp32 end to end.
"""

import os
import sys

for _p in ("/opt/trn_rl_repo", "/root/.axon_site/_ro/trn_rl_repo"):
    if os.path.isdir(_p) and _p not in sys.path:
        sys.path.insert(0, _p)

import numpy as np
import ml_dtypes
from contextlib import ExitStack

import concourse.bass as bass
import concourse.mybir as mybir
import concourse.tile as tile
from concourse import bacc
from concourse.bass_utils import run_bass_kernel_spmd

F32 = mybir.dt.float32
BF16 = mybir.dt.bfloat16
AF = mybir.ActivationFunctionType

H = 128
C_E = 384
B, N, K = 2, 4096, 48
SCALE = 30.0
N_CORES = 8
NODES = B * N // N_CORES          # 1024 nodes per core
TOK = NODES * K                   # 49152 edge tokens per core
CH_NODES = 16                     # nodes per chunk
CH_TOK = CH_NODES * K             # 768 tokens per chunk
N_CH = NODES // CH_NODES          # 64 chunks
G_LD = CH_TOK // 128              # 6 load tiles of 128 tokens per chunk
HALF = CH_TOK // 2                # 384-wide matmul halves

_CACHE = {}


def _build(debug_taps=False):
    nc = bacc.Bacc("TRN2", target_bir_lowering=False, debug=False)

    hE = nc.declare_dram_parameter("hE", [TOK, C_E], F32, isOutput=False)
    hV = nc.declare_dram_parameter("hV", [NODES, H], F32, isOutput=False)
    maskA = nc.declare_dram_parameter("maskA", [1, TOK], BF16, isOutput=False)
    maskAT = nc.declare_dram_parameter("maskAT", [K, NODES], BF16, isOutput=False)
    maskV = nc.declare_dram_parameter("maskV", [1, NODES], F32, isOutput=False)
    W1v = nc.declare_dram_parameter("W1v", [128, H], BF16, isOutput=False)
    W1e = nc.declare_dram_parameter("W1e", [128, 3, H], BF16, isOutput=False)
    W2 = nc.declare_dram_parameter("W2", [128, H], BF16, isOutput=False)
    W3s = nc.declare_dram_parameter("W3s", [128, H], BF16, isOutput=False)
    b1 = nc.declare_dram_parameter("b1", [128, 1], F32, isOutput=False)
    b2 = nc.declare_dram_parameter("b2", [128, 1], F32, isOutput=False)
    b3srow = nc.declare_dram_parameter("b3srow", [1, 128], BF16, isOutput=False)
    Win = nc.declare_dram_parameter("Win", [128, 4, 128], BF16, isOutput=False)
    Winb = nc.declare_dram_parameter("Winb", [128, 4], F32, isOutput=False)
    Wout = nc.declare_dram_parameter("Wout", [128, 4, 128], BF16, isOutput=False)
    boutrow = nc.declare_dram_parameter("boutrow", [1, 128], BF16, isOutput=False)
    ones_bf = nc.declare_dram_parameter("ones_bf", [1, 128], BF16, isOutput=False)
    ones_f = nc.declare_dram_parameter("ones_f", [1, 128], F32, isOutput=False)
    ones48 = nc.declare_dram_parameter("ones48", [K, 1], BF16, isOutput=False)
    onesN = nc.declare_dram_parameter("onesN", [1, 512], BF16, isOutput=False)
    ident = nc.declare_dram_parameter("ident", [128, 128], F32, isOutput=False)

    OUT = nc.declare_dram_parameter("OUT", [NODES, H], F32, isOutput=True)
    if debug_taps:
        DBG_VT = nc.declare_dram_parameter("DBG_VT", [128, NODES], F32, isOutput=True)
        DBG_S = nc.declare_dram_parameter("DBG_S", [128, NODES], F32, isOutput=True)
        DBG_HV1 = nc.declare_dram_parameter("DBG_HV1", [128, NODES], F32, isOutput=True)
        DBG_OT = nc.declare_dram_parameter("DBG_OT", [128, NODES], F32, isOutput=True)
        DBG_XT = nc.declare_dram_parameter("DBG_XT", [128, 3, CH_TOK], F32, isOutput=True)
        DBG_H1 = nc.declare_dram_parameter("DBG_H1", [128, CH_TOK], F32, isOutput=True)
        DBG_HM = nc.declare_dram_parameter("DBG_HM", [128, CH_TOK], F32, isOutput=True)

    with tile.TileContext(nc) as tc, ExitStack() as ctx:
        wp = ctx.enter_context(tc.tile_pool(name="wp", bufs=1))
        acc = ctx.enter_context(tc.tile_pool(name="acc", bufs=1))

        # ---- weights / constants to SBUF
        W1v_sb = wp.tile([128, H], BF16)
        nc.gpsimd.dma_start(out=W1v_sb[:], in_=W1v[:])
        W1e_sb = wp.tile([128, 3, H], BF16)
        nc.gpsimd.dma_start(out=W1e_sb[:], in_=W1e[:])
        W2_sb = wp.tile([128, H], BF16)
        nc.gpsimd.dma_start(out=W2_sb[:], in_=W2[:])
        W3s_sb = wp.tile([128, H], BF16)
        nc.gpsimd.dma_start(out=W3s_sb[:], in_=W3s[:])
        b1_sb = wp.tile([128, 1], F32)
        nc.gpsimd.dma_start(out=b1_sb[:], in_=b1[:])
        b2_sb = wp.tile([128, 1], F32)
        nc.gpsimd.dma_start(out=b2_sb[:], in_=b2[:])
        b3s_sb = wp.tile([1, 128], BF16)
        nc.gpsimd.dma_start(out=b3s_sb[:], in_=b3srow[:])
        Win_sb = wp.tile([128, 4, 128], BF16)
        nc.gpsimd.dma_start(out=Win_sb[:], in_=Win[:])
        Winb_sb = wp.tile([128, 4], F32)
        nc.gpsimd.dma_start(out=Winb_sb[:], in_=Winb[:])
        Wout_sb = wp.tile([128, 4, 128], BF16)
        nc.gpsimd.dma_start(out=Wout_sb[:], in_=Wout[:])
        bout_sb = wp.tile([1, 128], BF16)
        nc.gpsimd.dma_start(out=bout_sb[:], in_=boutrow[:])
        ones_bf_sb = wp.tile([1, 128], BF16)
        nc.gpsimd.dma_start(out=ones_bf_sb[:], in_=ones_bf[:])
        ones_f_sb = wp.tile([1, 128], F32)
        nc.gpsimd.dma_start(out=ones_f_sb[:], in_=ones_f[:])
        ones48_sb = wp.tile([K, 1], BF16)
        nc.gpsimd.dma_start(out=ones48_sb[:], in_=ones48[:])
        onesN_sb = wp.tile([1, 512], BF16)
        nc.gpsimd.dma_start(out=onesN_sb[:], in_=onesN[:])
        ident_sb = wp.tile([128, 128], F32)
        nc.gpsimd.dma_start(out=ident_sb[:], in_=ident[:])
        maskAT_sb = wp.tile([K, NODES], BF16)
        nc.gpsimd.dma_start(out=maskAT_sb[:], in_=maskAT[:])
        maskV_sb = wp.tile([1, NODES], F32)
        nc.gpsimd.dma_start(out=maskV_sb[:], in_=maskV[:])
        maskA_sb = wp.tile([1, TOK], BF16)
        nc.gpsimd.dma_start(out=maskA_sb[:], in_=maskA[:])

        hV_sb = wp.tile([128, NODES // 128, H], F32)
        nc.gpsimd.dma_start(out=hV_sb[:], in_=hV.rearrange("(t p) h -> p t h", p=128))

        # ---- h_V transpose: VT [H, NODES] in f32 (residual) and bf16 (matmul)
        VT_f = acc.tile([128, NODES], F32)
        VT_bf = acc.tile([128, NODES], BF16)
        S_f = acc.tile([128, NODES], F32)

        with tc.tile_pool(name="pst", bufs=2, space="PSUM") as pst:
            for t in range(NODES // 128):
                ps_t = pst.tile([128, 128], F32)
                nc.tensor.transpose(ps_t[:], hV_sb[:, t, :], ident_sb[:])
                nc.vector.tensor_copy(VT_f[:, 128 * t:128 * (t + 1)], ps_t[:])
                nc.scalar.copy(VT_bf[:, 128 * t:128 * (t + 1)], ps_t[:])

        # ---- edge phase
        with (
            tc.tile_pool(name="lp", bufs=3) as lp,
            tc.tile_pool(name="xp", bufs=3) as xp,
            tc.tile_pool(name="hp", bufs=2) as hp,
            tc.tile_pool(name="pp1", bufs=2, space="PSUM") as pp1,
            tc.tile_pool(name="pp2", bufs=1, space="PSUM") as pp2,
            tc.tile_pool(name="ppm", bufs=1, space="PSUM") as ppm,
        ):
            for c in range(N_CH):
                tok0 = c * CH_TOK
                hE_t = lp.tile([128, G_LD, C_E], BF16)
                nc.gpsimd.dma_start(
                    out=hE_t[:],
                    in_=hE[tok0:tok0 + CH_TOK, :].rearrange("(g p) c -> p g c", p=128),
                )
                xT = xp.tile([128, 3, CH_TOK], BF16)
                for j in range(3):
                    for g in range(G_LD):
                        nc.sync.dma_start(
                            out=xT[:, j, 128 * g:128 * (g + 1)],
                            in_=hE_t[:, g, 128 * j:128 * (j + 1)],
                            transpose=True,
                        )

                # psum tiles are [128, 1024] with the two 384-wide halves at
                # offsets 0 and 512 so each matmul output stays in one bank
                psumM = ppm.tile([128, 2, 512], F32)
                for h in range(2):
                    nc.tensor.matmul(
                        psumM[:, h, :HALF],
                        ones_bf_sb[:],
                        maskA_sb[0:1, tok0 + HALF * h:tok0 + HALF * (h + 1)],
                        start=True, stop=True,
                    )

                psum1 = pp1.tile([128, 2, 512], F32)
                for h in range(2):
                    for j in range(3):
                        nc.tensor.matmul(
                            psum1[:, h, :HALF], W1e_sb[:, j, :],
                            xT[:, j, HALF * h:HALF * (h + 1)],
                            start=(j == 0), stop=False,
                        )
                    n0 = c * CH_NODES + 8 * h
                    nc.tensor.matmul(
                        psum1[:, h, :HALF].rearrange("p (g k) -> p g k", k=K),
                        W1v_sb[:],
                        VT_bf[:, n0:n0 + 8, None].to_broadcast([128, 8, K]),
                        start=False, stop=True,
                    )

                h1g = hp.tile([128, CH_TOK], BF16)
                h1g_v = h1g[:].rearrange("p (h x) -> p h x", h=2)
                nc.scalar.activation(h1g_v, psum1[:, :, :HALF],
                                     AF.Gelu_apprx_tanh,
                                     bias=b1_sb[:], scale=1.0)

                psum2 = pp2.tile([128, 2, 512], F32)
                for h in range(2):
                    nc.tensor.matmul(psum2[:, h, :HALF], W2_sb[:],
                                     h1g[:, HALF * h:HALF * (h + 1)],
                                     start=True, stop=True)

                h2g = hp.tile([128, CH_TOK], BF16)
                h2g_v = h2g[:].rearrange("p (h x) -> p h x", h=2)
                nc.scalar.activation(h2g_v, psum2[:, :, :HALF],
                                     AF.Gelu_apprx_tanh,
                                     bias=b2_sb[:], scale=1.0)

                hm = hp.tile([128, CH_TOK], BF16)
                nc.vector.tensor_tensor(hm[:].rearrange("p (h x) -> p h x", h=2),
                                        h2g_v, psumM[:, :, :HALF],
                                        mybir.AluOpType.mult)
                nc.vector.tensor_reduce(
                    S_f[:, c * CH_NODES:(c + 1) * CH_NODES],
                    hm[:].rearrange("p (g k) -> p g k", k=K),
                    mybir.AxisListType.X, mybir.AluOpType.add,
                )
                if debug_taps and c == 0:
                    xtf = hp.tile([128, 3, CH_TOK], F32, tag="dbgxt", bufs=1)
                    nc.vector.tensor_copy(xtf[:], xT[:])
                    nc.gpsimd.dma_start(out=DBG_XT[:], in_=xtf[:])
                    h1f = hp.tile([128, CH_TOK], F32, tag="dbgh1", bufs=1)
                    nc.vector.tensor_copy(h1f[:], h1g[:])
                    nc.gpsimd.dma_start(out=DBG_H1[:], in_=h1f[:])
                    hmf = hp.tile([128, CH_TOK], F32, tag="dbghm", bufs=1)
                    nc.vector.tensor_copy(hmf[:], hm[:])
                    nc.gpsimd.dma_start(out=DBG_HM[:], in_=hmf[:])

        # ---- node phase
        S_bf = acc.tile([128, NODES], BF16)
        nc.vector.tensor_copy(S_bf[:], S_f[:])

        hv1_f = acc.tile([128, NODES], F32)
        hv1_bf = acc.tile([128, NODES], BF16)
        outT_f = acc.tile([128, NODES], F32)
        outN_sb = acc.tile([128, NODES // 128, H], F32)

        with tc.tile_pool(name="np1", bufs=1, space="PSUM") as np1:
            psA = np1.tile([1, NODES], F32)
            for h in range(2):
                nc.tensor.matmul(psA[0:1, 512 * h:512 * (h + 1)], ones48_sb[:],
                                 maskAT_sb[:, 512 * h:512 * (h + 1)],
                                 start=True, stop=True)
            msum_bf = acc.tile([1, NODES], BF16)
            nc.vector.tensor_copy(msum_bf[:], psA[:])

            psum_dh = np1.tile([128, NODES], F32)
            for h in range(2):
                sl = slice(512 * h, 512 * (h + 1))
                nc.tensor.matmul(psum_dh[:, sl], W3s_sb[:], S_bf[:, sl],
                                 start=True, stop=False)
                nc.tensor.matmul(psum_dh[:, sl], b3s_sb[:], msum_bf[0:1, sl],
                                 start=False, stop=True)
            nc.vector.tensor_tensor(hv1_f[:], VT_f[:], psum_dh[:],
                                    mybir.AluOpType.add)
            nc.vector.tensor_copy(hv1_bf[:], hv1_f[:])

        with tc.tile_pool(name="np2", bufs=1, space="PSUM") as np2:
            for nh in range(2):
                sl = slice(512 * nh, 512 * (nh + 1))
                gqs = []
                for q in range(4):
                    psg = np2.tile([128, 512], F32, tag=f"psg{q}")
                    nc.tensor.matmul(psg[:], Win_sb[:, q, :], hv1_bf[:, sl],
                                     start=True, stop=True)
                    gq = acc.tile([128, 512], BF16, tag=f"gq{q}", bufs=2)
                    nc.scalar.activation(gq[:], psg[:], AF.Gelu,
                                         bias=Winb_sb[:, q:q + 1], scale=1.0)
                    gqs.append(gq)
                pso = np2.tile([128, 512], F32, tag="pso")
                for q in range(4):
                    nc.tensor.matmul(pso[:], Wout_sb[:, q, :], gqs[q][:],
                                     start=(q == 0), stop=False)
                nc.tensor.matmul(pso[:], bout_sb[:], onesN_sb[:],
                                 start=False, stop=True)
                psmv = np2.tile([128, 512], F32, tag="psmv")
                nc.tensor.matmul(psmv[:], ones_f_sb[:], maskV_sb[0:1, sl],
                                 start=True, stop=True)
                o1 = acc.tile([128, 512], F32, tag="o1", bufs=2)
                nc.vector.tensor_tensor(o1[:], hv1_f[:, sl], pso[:],
                                        mybir.AluOpType.add)
                nc.vector.tensor_tensor(outT_f[:, sl], o1[:], psmv[:],
                                        mybir.AluOpType.mult)

        with tc.tile_pool(name="np3", bufs=2, space="PSUM") as np3:
            for t in range(NODES // 128):
                ps_t = np3.tile([128, 128], F32)
                nc.tensor.transpose(ps_t[:], outT_f[:, 128 * t:128 * (t + 1)],
                                    ident_sb[:])
                nc.vector.tensor_copy(outN_sb[:, t, :], ps_t[:])

        nc.gpsimd.dma_start(out=OUT.rearrange("(t p) h -> p t h", p=128),
                          in_=outN_sb[:])
        if debug_taps:
            nc.gpsimd.dma_start(out=DBG_VT[:], in_=VT_f[:])
            nc.gpsimd.dma_start(out=DBG_S[:], in_=S_f[:])
            nc.gpsimd.dma_start(out=DBG_HV1[:], in_=hv1_f[:])
            nc.gpsimd.dma_start(out=DBG_OT[:], in_=outT_f[:])

    nc.compile()
    return nc


def _get_program():
    if "nc" not in _CACHE:
        _CACHE["nc"] = _build()
    return _CACHE["nc"]


def _prep_core_inputs(h_V, h_E, mask_V, mask_attend, W1_w, W1_b, W2_w, W2_b,
                      W3_w, W3_b, Win_w, Win_b, Wout_w, Wout_b):
    bf = ml_dtypes.bfloat16
    shared = dict(
        W1v=np.ascontiguousarray(W1_w[:128]).astype(bf),
        W1e=np.ascontiguousarray(
            W1_w[128:].reshape(3, 128, H).transpose(1, 0, 2)).astype(bf),
        W2=W2_w.astype(bf),
        W3s=(W3_w / SCALE).astype(bf),
        b1=np.asarray(W1_b, np.float32).reshape(128, 1),
        b2=np.asarray(W2_b, np.float32).reshape(128, 1),
        b3srow=(np.asarray(W3_b, np.float32) / SCALE).reshape(1, 128).astype(bf),
        Win=np.ascontiguousarray(
            Win_w.reshape(H, 4, 128).transpose(0, 1, 2)).astype(bf),
        Winb=np.ascontiguousarray(
            np.asarray(Win_b, np.float32).reshape(4, 128).T),
        Wout=np.ascontiguousarray(
            Wout_w.reshape(4, 128, H).transpose(1, 0, 2)).astype(bf),
        boutrow=np.asarray(Wout_b, np.float32).reshape(1, 128).astype(bf),
        ones_bf=np.ones((1, 128), bf),
        ones_f=np.ones((1, 128), np.float32),
        ones48=np.ones((K, 1), bf),
        onesN=np.ones((1, 512), bf),
        ident=np.eye(128, dtype=np.float32),
    )

    hV_all = np.asarray(h_V, np.float32).reshape(B * N, H)
    hE_all = np.asarray(h_E, np.float32).reshape(B * N, K, C_E)
    mA_all = np.asarray(mask_attend, np.float32).reshape(B * N, K)
    mV_all = np.asarray(mask_V, np.float32).reshape(B * N)

    in_maps = []
    for i in range(N_CORES):
        s = slice(i * NODES, (i + 1) * NODES)
        in_maps.append(dict(
            hE=np.ascontiguousarray(hE_all[s].reshape(TOK, C_E)),
            hV=np.ascontiguousarray(hV_all[s]),
            maskA=np.ascontiguousarray(mA_all[s].reshape(1, TOK)).astype(bf),
            maskAT=np.ascontiguousarray(mA_all[s].T).astype(bf),
            maskV=np.ascontiguousarray(mV_all[s].reshape(1, NODES)),
            **shared,
        ))
    return in_maps


def kernel(**inputs) -> np.ndarray:
    nc = _get_program()
    in_maps = _prep_core_inputs(**inputs)
    res = run_bass_kernel_spmd(nc, in_maps, list(range(N_CORES)))
    out = np.concatenate([np.asarray(r["OUT"], np.float32)
                          for r in res.results], axis=0)
    return out.reshape(B, N, H)


# revision 30
# speedup vs baseline: 199.9294x; 199.9294x over previous
"""Trainium2 Bass kernel for nn_DecLayerJ (gnn message passing decoder layer).

Strategy (per NeuronCore, 8-way data parallel over B*N nodes):
  - Edge phase: stream h_E in 768-token chunks (16 nodes x K=48).
    SWDGE cast-load f32->bf16, xbar-DMA transpose to feature-major
    [C, tokens], W1/W2 matmuls in bf16 (f32 PSUM accum), tanh-gelu on ACT,
    mask broadcast via rank-1 PE matmul, masked K-sum via DVE reduce.
  - Node phase: S @ (W3/30) + (sum_k mask)*b3/30, residual, FFN with exact
    gelu, mask_V, transpose back to token-major, store.
h_V residual path stays fp32 end to end.
"""

import os
import sys

for _p in ("/opt/trn_rl_repo", "/root/.axon_site/_ro/trn_rl_repo"):
    if os.path.isdir(_p) and _p not in sys.path:
        sys.path.insert(0, _p)

import numpy as np
import ml_dtypes
from contextlib import ExitStack

import concourse.bass as bass
import concourse.mybir as mybir
import concourse.tile as tile
from concourse import bacc
from concourse.bass_utils import run_bass_kernel_spmd

F32 = mybir.dt.float32
BF16 = mybir.dt.bfloat16
AF = mybir.ActivationFunctionType

H = 128
C_E = 384
B, N, K = 2, 4096, 48
SCALE = 30.0
N_CORES = 8
NODES = B * N // N_CORES          # 1024 nodes per core
TOK = NODES * K                   # 49152 edge tokens per core
CH_NODES = 16                     # nodes per chunk
CH_TOK = CH_NODES * K             # 768 tokens per chunk
N_CH = NODES // CH_NODES          # 64 chunks
G_LD = CH_TOK // 128              # 6 128-token groups per chunk
HALF = CH_TOK // 2                # 384-wide matmul halves
SUPER = 4                         # chunks per super-chunk (one load+xpose each)
N_SUP = N_CH // SUPER             # 16 super-chunks
SUP_TOK = SUPER * CH_TOK          # 3072 tokens
G_SUP = SUP_TOK // 128            # 24 128-token groups per super-chunk

_CACHE = {}


def _build(debug_taps=False, n_ch=N_CH, variant=None):
    nc = bacc.Bacc("TRN2", target_bir_lowering=False, debug=False)

    hE = nc.declare_dram_parameter("hE", [TOK, C_E], F32, isOutput=False)
    hV = nc.declare_dram_parameter("hV", [NODES, H], F32, isOutput=False)
    maskA = nc.declare_dram_parameter("maskA", [1, TOK], BF16, isOutput=False)
    maskAT = nc.declare_dram_parameter("maskAT", [K, NODES], BF16, isOutput=False)
    maskV = nc.declare_dram_parameter("maskV", [1, NODES], F32, isOutput=False)
    W1v = nc.declare_dram_parameter("W1v", [128, H], BF16, isOutput=False)
    W1e = nc.declare_dram_parameter("W1e", [128, 3, H], BF16, isOutput=False)
    W2 = nc.declare_dram_parameter("W2", [128, H], BF16, isOutput=False)
    W3s = nc.declare_dram_parameter("W3s", [128, H], BF16, isOutput=False)
    b1 = nc.declare_dram_parameter("b1", [128, 1], F32, isOutput=False)
    b2 = nc.declare_dram_parameter("b2", [128, 1], F32, isOutput=False)
    b3srow = nc.declare_dram_parameter("b3srow", [1, 128], BF16, isOutput=False)
    Win = nc.declare_dram_parameter("Win", [128, 4, 128], BF16, isOutput=False)
    Winb = nc.declare_dram_parameter("Winb", [128, 4], F32, isOutput=False)
    Wout = nc.declare_dram_parameter("Wout", [128, 4, 128], BF16, isOutput=False)
    boutrow = nc.declare_dram_parameter("boutrow", [1, 128], BF16, isOutput=False)
    ones_bf = nc.declare_dram_parameter("ones_bf", [1, 128], BF16, isOutput=False)
    ones_f = nc.declare_dram_parameter("ones_f", [1, 128], F32, isOutput=False)
    ones48 = nc.declare_dram_parameter("ones48", [K, 1], BF16, isOutput=False)
    onesN = nc.declare_dram_parameter("onesN", [1, 512], BF16, isOutput=False)
    ident = nc.declare_dram_parameter("ident", [128, 128], F32, isOutput=False)

    OUT = nc.declare_dram_parameter("OUT", [NODES, H], F32, isOutput=True)
    if debug_taps:
        DBG_VT = nc.declare_dram_parameter("DBG_VT", [128, NODES], F32, isOutput=True)
        DBG_S = nc.declare_dram_parameter("DBG_S", [128, NODES], F32, isOutput=True)
        DBG_HV1 = nc.declare_dram_parameter("DBG_HV1", [128, NODES], F32, isOutput=True)
        DBG_OT = nc.declare_dram_parameter("DBG_OT", [128, NODES], F32, isOutput=True)
        DBG_XT = nc.declare_dram_parameter("DBG_XT", [128, 3, CH_TOK], F32, isOutput=True)
        DBG_H1 = nc.declare_dram_parameter("DBG_H1", [128, CH_TOK], F32, isOutput=True)
        DBG_HM = nc.declare_dram_parameter("DBG_HM", [128, CH_TOK], F32, isOutput=True)

    with tile.TileContext(nc) as tc, ExitStack() as ctx:
        wp = ctx.enter_context(tc.tile_pool(name="wp", bufs=1))
        acc = ctx.enter_context(tc.tile_pool(name="acc", bufs=1))

        # ---- weights / constants to SBUF
        W1v_sb = wp.tile([128, H], BF16)
        nc.gpsimd.dma_start(out=W1v_sb[:], in_=W1v[:])
        W1e_sb = wp.tile([128, 3, H], BF16)
        nc.gpsimd.dma_start(out=W1e_sb[:], in_=W1e[:])
        W2_sb = wp.tile([128, H], BF16)
        nc.gpsimd.dma_start(out=W2_sb[:], in_=W2[:])
        W3s_sb = wp.tile([128, H], BF16)
        nc.gpsimd.dma_start(out=W3s_sb[:], in_=W3s[:])
        b1_sb = wp.tile([128, 1], F32)
        nc.gpsimd.dma_start(out=b1_sb[:], in_=b1[:])
        b2_sb = wp.tile([128, 1], F32)
        nc.gpsimd.dma_start(out=b2_sb[:], in_=b2[:])
        b3s_sb = wp.tile([1, 128], BF16)
        nc.gpsimd.dma_start(out=b3s_sb[:], in_=b3srow[:])
        Win_sb = wp.tile([128, 4, 128], BF16)
        nc.gpsimd.dma_start(out=Win_sb[:], in_=Win[:])
        Winb_sb = wp.tile([128, 4], F32)
        nc.gpsimd.dma_start(out=Winb_sb[:], in_=Winb[:])
        Wout_sb = wp.tile([128, 4, 128], BF16)
        nc.gpsimd.dma_start(out=Wout_sb[:], in_=Wout[:])
        bout_sb = wp.tile([1, 128], BF16)
        nc.gpsimd.dma_start(out=bout_sb[:], in_=boutrow[:])
        ones_bf_sb = wp.tile([1, 128], BF16)
        nc.gpsimd.dma_start(out=ones_bf_sb[:], in_=ones_bf[:])
        ones_f_sb = wp.tile([1, 128], F32)
        nc.gpsimd.dma_start(out=ones_f_sb[:], in_=ones_f[:])
        ones48_sb = wp.tile([K, 1], BF16)
        nc.gpsimd.dma_start(out=ones48_sb[:], in_=ones48[:])
        onesN_sb = wp.tile([1, 512], BF16)
        nc.gpsimd.dma_start(out=onesN_sb[:], in_=onesN[:])
        ident_sb = wp.tile([128, 128], F32)
        nc.gpsimd.dma_start(out=ident_sb[:], in_=ident[:])
        maskAT_sb = wp.tile([K, NODES], BF16)
        nc.gpsimd.dma_start(out=maskAT_sb[:], in_=maskAT[:])
        maskV_sb = wp.tile([1, NODES], F32)
        nc.gpsimd.dma_start(out=maskV_sb[:], in_=maskV[:])

        hV_sb = wp.tile([128, NODES // 128, H], F32)
        nc.gpsimd.dma_start(out=hV_sb[:], in_=hV.rearrange("(t p) h -> p t h", p=128))

        # ---- h_V transpose: VT [H, NODES] in f32 (residual) and bf16 (matmul)
        VT_f = acc.tile([128, NODES], F32)
        VT_bf = acc.tile([128, NODES], BF16)
        S_f = acc.tile([128, NODES], F32)

        with tc.tile_pool(name="pst", bufs=2, space="PSUM") as pst:
            for t in range(NODES // 128):
                ps_t = pst.tile([128, 128], F32)
                nc.tensor.transpose(ps_t[:], hV_sb[:, t, :], ident_sb[:])
                nc.vector.tensor_copy(VT_f[:, 128 * t:128 * (t + 1)], ps_t[:])
                nc.scalar.copy(VT_bf[:, 128 * t:128 * (t + 1)], ps_t[:])

        # ---- edge phase: super-chunks of 4 chunks; one cast-load + one
        # batched xbar transpose + one mask load per super-chunk
        with (
            tc.tile_pool(name="lp", bufs=3) as lp,
            tc.tile_pool(name="xp", bufs=3) as xp,
            tc.tile_pool(name="mp", bufs=2) as mp,
            tc.tile_pool(name="hp", bufs=3) as hp,
            tc.tile_pool(name="pp1", bufs=2, space="PSUM") as pp1,
            tc.tile_pool(name="pp2", bufs=1, space="PSUM") as pp2,
            tc.tile_pool(name="ppm", bufs=1, space="PSUM") as ppm,
        ):
            for s in range(n_ch // SUPER):
                stok0 = s * SUP_TOK
                # host pre-permutes hE rows so each partition reads one
                # contiguous 24-row (36 KB) run: dev row 24p+g -> hE_t[p, g]
                hE_t = lp.tile([128, G_SUP, C_E], BF16)
                nc.gpsimd.dma_start(
                    out=hE_t[:],
                    in_=hE[stok0:stok0 + SUP_TOK, :].rearrange(
                        "(p g) c -> p g c", g=G_SUP),
                )
                mA_t = mp.tile([1, SUP_TOK], BF16)
                nc.gpsimd.dma_start(out=mA_t[:],
                                    in_=maskA[0:1, stok0:stok0 + SUP_TOK])
                # one batched xbar transpose per super-chunk:
                # xT2[c', g, j, t] = hE_t[t, g, 128*j + c']
                xT2 = xp.tile([128, G_SUP, 3, 128], BF16)
                nc.sync.dma_start(out=xT2[:], in_=hE_t[:], transpose=True)

                for cc in range(SUPER):
                    c = s * SUPER + cc
                    # psum tiles are [128, 1024]: the two 384-wide halves sit
                    # at offsets 0 and 512 so each matmul stays in one bank
                    psum1 = pp1.tile([128, 2, 512], F32)
                    for h in range(2):
                        g0 = 6 * cc + 3 * h
                        for j in range(3):
                            nc.tensor.matmul(
                                psum1[:, h, :HALF], W1e_sb[:, j, :],
                                xT2[:, g0:g0 + 3, j, :],
                                start=(j == 0), stop=False,
                            )
                        n0 = c * CH_NODES + 8 * h
                        nc.tensor.matmul(
                            psum1[:, h, :HALF].rearrange("p (g k) -> p g k", k=K),
                            W1v_sb[:],
                            VT_bf[:, n0:n0 + 8, None].to_broadcast([128, 8, K]),
                            start=False, stop=True,
                        )

                    h1g = hp.tile([128, CH_TOK], BF16)
                    h1g_v = h1g[:].rearrange("p (h x) -> p h x", h=2)
                    nc.scalar.activation(h1g_v, psum1[:, :, :HALF],
                                         AF.Gelu_apprx_tanh,
                                         bias=b1_sb[:], scale=1.0)

                    psum2 = pp2.tile([128, 2, 512], F32)
                    for h in range(2):
                        nc.tensor.matmul(psum2[:, h, :HALF], W2_sb[:],
                                         h1g[:, HALF * h:HALF * (h + 1)],
                                         start=True, stop=True)

                    # mask broadcast late in the PE stream: its psum slot is
                    # only freed by the previous chunk's DVE mul
                    psumM = ppm.tile([128, 2, 512], F32)
                    for h in range(2):
                        nc.tensor.matmul(
                            psumM[:, h, :HALF],
                            ones_bf_sb[:],
                            mA_t[0:1, cc * CH_TOK + HALF * h:
                                 cc * CH_TOK + HALF * (h + 1)],
                            start=True, stop=True,
                        )

                    h2g = hp.tile([128, CH_TOK], BF16)
                    h2g_v = h2g[:].rearrange("p (h x) -> p h x", h=2)
                    nc.scalar.activation(h2g_v, psum2[:, :, :HALF],
                                         AF.Gelu_apprx_tanh,
                                         bias=b2_sb[:], scale=1.0)

                    hm = hp.tile([128, CH_TOK], BF16)
                    nc.vector.tensor_tensor(
                        hm[:].rearrange("p (h x) -> p h x", h=2),
                        h2g_v, psumM[:, :, :HALF], mybir.AluOpType.mult)
                    nc.vector.tensor_reduce(
                        S_f[:, c * CH_NODES:(c + 1) * CH_NODES],
                        hm[:].rearrange("p (g k) -> p g k", k=K),
                        mybir.AxisListType.X, mybir.AluOpType.add,
                    )
                    if debug_taps and c == 0:
                        xtf = hp.tile([128, 3, CH_TOK], F32, tag="dbgxt", bufs=1)
                        for j in range(3):
                            nc.vector.tensor_copy(
                                xtf[:, j, :].rearrange("p (g t) -> p g t", t=128),
                                xT2[:, :G_LD, j, :])
                        nc.gpsimd.dma_start(out=DBG_XT[:], in_=xtf[:])
                        h1f = hp.tile([128, CH_TOK], F32, tag="dbgh1", bufs=1)
                        nc.vector.tensor_copy(h1f[:], h1g[:])
                        nc.gpsimd.dma_start(out=DBG_H1[:], in_=h1f[:])
                        hmf = hp.tile([128, CH_TOK], F32, tag="dbghm", bufs=1)
                        nc.vector.tensor_copy(hmf[:], hm[:])
                        nc.gpsimd.dma_start(out=DBG_HM[:], in_=hmf[:])

        # ---- node phase
        S_bf = acc.tile([128, NODES], BF16)
        nc.vector.tensor_copy(S_bf[:], S_f[:])

        hv1_f = acc.tile([128, NODES], F32)
        hv1_bf = acc.tile([128, NODES], BF16)
        outT_f = acc.tile([128, NODES], F32)
        outN_sb = acc.tile([128, NODES // 128, H], F32)

        with tc.tile_pool(name="np1", bufs=1, space="PSUM") as np1:
            psA = np1.tile([1, NODES], F32)
            for h in range(2):
                nc.tensor.matmul(psA[0:1, 512 * h:512 * (h + 1)], ones48_sb[:],
                                 maskAT_sb[:, 512 * h:512 * (h + 1)],
                                 start=True, stop=True)
            msum_bf = acc.tile([1, NODES], BF16)
            nc.vector.tensor_copy(msum_bf[:], psA[:])

            psum_dh = np1.tile([128, NODES], F32)
            for h in range(2):
                sl = slice(512 * h, 512 * (h + 1))
                nc.tensor.matmul(psum_dh[:, sl], W3s_sb[:], S_bf[:, sl],
                                 start=True, stop=False)
                nc.tensor.matmul(psum_dh[:, sl], b3s_sb[:], msum_bf[0:1, sl],
                                 start=False, stop=True)
            nc.vector.tensor_tensor(hv1_f[:], VT_f[:], psum_dh[:],
                                    mybir.AluOpType.add)
            nc.vector.tensor_copy(hv1_bf[:], hv1_f[:])

        with tc.tile_pool(name="np2", bufs=1, space="PSUM") as np2:
            for nh in range(2):
                sl = slice(512 * nh, 512 * (nh + 1))
                gqs = []
                for q in range(4):
                    psg = np2.tile([128, 512], F32, tag=f"psg{q}")
                    nc.tensor.matmul(psg[:], Win_sb[:, q, :], hv1_bf[:, sl],
                                     start=True, stop=True)
                    gq = acc.tile([128, 512], BF16, tag=f"gq{q}", bufs=2)
                    nc.scalar.activation(gq[:], psg[:], AF.Gelu,
                                         bias=Winb_sb[:, q:q + 1], scale=1.0)
                    gqs.append(gq)
                pso = np2.tile([128, 512], F32, tag="pso")
                for q in range(4):
                    nc.tensor.matmul(pso[:], Wout_sb[:, q, :], gqs[q][:],
                                     start=(q == 0), stop=False)
                nc.tensor.matmul(pso[:], bout_sb[:], onesN_sb[:],
                                 start=False, stop=True)
                psmv = np2.tile([128, 512], F32, tag="psmv")
                nc.tensor.matmul(psmv[:], ones_f_sb[:], maskV_sb[0:1, sl],
                                 start=True, stop=True)
                o1 = acc.tile([128, 512], F32, tag="o1", bufs=2)
                nc.vector.tensor_tensor(o1[:], hv1_f[:, sl], pso[:],
                                        mybir.AluOpType.add)
                nc.vector.tensor_tensor(outT_f[:, sl], o1[:], psmv[:],
                                        mybir.AluOpType.mult)

        with tc.tile_pool(name="np3", bufs=2, space="PSUM") as np3:
            for t in range(NODES // 128):
                ps_t = np3.tile([128, 128], F32)
                nc.tensor.transpose(ps_t[:], outT_f[:, 128 * t:128 * (t + 1)],
                                    ident_sb[:])
                nc.vector.tensor_copy(outN_sb[:, t, :], ps_t[:])

        nc.gpsimd.dma_start(out=OUT.rearrange("(t p) h -> p t h", p=128),
                          in_=outN_sb[:])
        if debug_taps:
            nc.gpsimd.dma_start(out=DBG_VT[:], in_=VT_f[:])
            nc.gpsimd.dma_start(out=DBG_S[:], in_=S_f[:])
            nc.gpsimd.dma_start(out=DBG_HV1[:], in_=hv1_f[:])
            nc.gpsimd.dma_start(out=DBG_OT[:], in_=outT_f[:])

    nc.compile()
    return nc


def _get_program():
    if "nc" not in _CACHE:
        _CACHE["nc"] = _build()
    return _CACHE["nc"]


def _prep_core_inputs(h_V, h_E, mask_V, mask_attend, W1_w, W1_b, W2_w, W2_b,
                      W3_w, W3_b, Win_w, Win_b, Wout_w, Wout_b):
    bf = ml_dtypes.bfloat16
    shared = dict(
        W1v=np.ascontiguousarray(W1_w[:128]).astype(bf),
        W1e=np.ascontiguousarray(
            W1_w[128:].reshape(3, 128, H).transpose(1, 0, 2)).astype(bf),
        W2=W2_w.astype(bf),
        W3s=(W3_w / SCALE).astype(bf),
        b1=np.asarray(W1_b, np.float32).reshape(128, 1),
        b2=np.asarray(W2_b, np.float32).reshape(128, 1),
        b3srow=(np.asarray(W3_b, np.float32) / SCALE).reshape(1, 128).astype(bf),
        Win=np.ascontiguousarray(
            Win_w.reshape(H, 4, 128).transpose(0, 1, 2)).astype(bf),
        Winb=np.ascontiguousarray(
            np.asarray(Win_b, np.float32).reshape(4, 128).T),
        Wout=np.ascontiguousarray(
            Wout_w.reshape(4, 128, H).transpose(1, 0, 2)).astype(bf),
        boutrow=np.asarray(Wout_b, np.float32).reshape(1, 128).astype(bf),
        ones_bf=np.ones((1, 128), bf),
        ones_f=np.ones((1, 128), np.float32),
        ones48=np.ones((K, 1), bf),
        onesN=np.ones((1, 512), bf),
        ident=np.eye(128, dtype=np.float32),
    )

    hV_all = np.asarray(h_V, np.float32).reshape(B * N, H)
    hE_all = np.asarray(h_E, np.float32).reshape(B * N, K, C_E)
    mA_all = np.asarray(mask_attend, np.float32).reshape(B * N, K)
    mV_all = np.asarray(mask_V, np.float32).reshape(B * N)

    # per-super-chunk row permutation so the device load AP "(p g) c" reads
    # one contiguous 24-row run per partition while matmul columns stay in
    # token order: dev[24p + 6cc + 3h + g'] = orig[768cc + 384h + 128g' + p]
    perm = np.empty(SUP_TOK, np.int64)
    for p in range(128):
        for cc in range(SUPER):
            for h in range(2):
                for g_ in range(3):
                    perm[24 * p + 6 * cc + 3 * h + g_] = (
                        768 * cc + 384 * h + 128 * g_ + p)

    in_maps = []
    for i in range(N_CORES):
        s = slice(i * NODES, (i + 1) * NODES)
        hE_core = hE_all[s].reshape(N_SUP, SUP_TOK, C_E)[:, perm, :]
        in_maps.append(dict(
            hE=np.ascontiguousarray(hE_core.reshape(TOK, C_E)),
            hV=np.ascontiguousarray(hV_all[s]),
            maskA=np.ascontiguousarray(mA_all[s].reshape(1, TOK)).astype(bf),
            maskAT=np.ascontiguousarray(mA_all[s].T).astype(bf),
            maskV=np.ascontiguousarray(mV_all[s].reshape(1, NODES)),
            **shared,
        ))
    return in_maps


def kernel(**inputs) -> np.ndarray:
    nc = _get_program()
    in_maps = _prep_core_inputs(**inputs)
    res = run_bass_kernel_spmd(nc, in_maps, list(range(N_CORES)))
    out = np.concatenate([np.asarray(r["OUT"], np.float32)
                          for r in res.results], axis=0)
    return out.reshape(B, N, H)
